# revision 1
# baseline (speedup 1.0000x reference)
"""H2HGCN message-passing kernel for 8 Trainium2 NeuronCores.

Self-contained: hardcodes problem shapes (N=30000, DEG=16, DIM=512, L=2),
shards nodes 8-way, runs one Bass/Tile NEFF per core via the PJRT path.
"""
import math
import sys

sys.path.insert(0, "/opt/trn_rl_repo")

import numpy as np

import concourse.bass as bass
import concourse.mybir as mybir
import concourse.tile as tile
from concourse import bacc
from concourse.bass import ds, ts
from concourse.masks import make_identity

# ---- problem constants ----
N, DEG, DIM, L = 30000, 16, 512, 2
NCORES = 8
NS = N // NCORES          # 3750 real nodes per core
P = 128
NT = (NS + P - 1) // P    # 30 tiles
NSP = NT * P              # 3840 padded nodes per core
NFULL = NCORES * NSP      # 30720 rows in the all-gathered z table
EPS = 1e-8
LAM = 1.0507009873554805
ALPHA = 1.6732632423543772
LA = LAM * ALPHA
LN_LA = math.log(LA)

f32 = mybir.dt.float32
f16 = mybir.dt.float16
i16 = mybir.dt.int16

A = mybir.AluOpType


def _build_nc(sim_mode=False):
    nc = bacc.Bacc("TRN2", target_bir_lowering=False, debug=False,
                   num_devices=1 if sim_mode else NCORES)
    ACT = mybir.ActivationFunctionType

    # ---- kernel I/O ----
    nrT = nc.dram_tensor("nrT", [513, NSP], f16, kind="ExternalInput")
    linwT = nc.dram_tensor("linwT", [513, DIM], f16, kind="ExternalInput")
    lw2 = nc.dram_tensor("lw2", [L, DIM, DIM], f16, kind="ExternalInput")
    wblk = nc.dram_tensor("wblk", [P, NT * 16 * 32], f16, kind="ExternalInput")
    idxs = nc.dram_tensor("idxs", [P, NT * P], i16, kind="ExternalInput")
    out_h = nc.dram_tensor("out_h", [NSP, DIM], f32, kind="ExternalOutput")

    with tile.TileContext(nc) as tc, tile.ExitStack() as ctx:
        consts = ctx.enter_context(tc.tile_pool(name="consts", bufs=1))
        nrt_pool = ctx.enter_context(tc.tile_pool(name="nrt", bufs=6))
        work = ctx.enter_context(tc.tile_pool(name="work", bufs=4))
        work2 = ctx.enter_context(tc.tile_pool(name="work2", bufs=4))
        zpool = ctx.enter_context(tc.tile_pool(name="zpool", bufs=4))
        gpool = ctx.enter_context(tc.tile_pool(name="gpool", bufs=3))
        scr_pool = ctx.enter_context(tc.tile_pool(name="scr", bufs=4))
        small = ctx.enter_context(tc.tile_pool(name="small", bufs=12))
        psum_mm = ctx.enter_context(tc.tile_pool(name="psum_mm", bufs=2, space="PSUM"))
        psum_nm = ctx.enter_context(tc.tile_pool(name="psum_nm", bufs=2, space="PSUM"))
        psum_tp = ctx.enter_context(tc.tile_pool(name="psum_tp", bufs=2, space="PSUM"))
        dram = ctx.enter_context(tc.tile_pool(name="dram", bufs=2, space="DRAM"))

        # ---- resident constants ----
        ident = consts.tile([P, P], f32)
        make_identity(nc, ident)
        c_lnla = consts.tile([P, 1], f32)
        nc.vector.memset(c_lnla, LN_LA)
        c_eps = consts.tile([P, 1], f32)
        nc.vector.memset(c_eps, EPS)
        c_one = consts.tile([P, 1], f32)
        nc.vector.memset(c_one, 1.0)

        linwT_sb = []
        for c in range(4):
            t = consts.tile([P, DIM], f16, tag=f"linwT{c}")
            nc.sync.dma_start(out=t, in_=linwT[ds(c * P, P), :])
            linwT_sb.append(t)
        linwT_b = consts.tile([1, DIM], f16, tag="linwTb")
        nc.sync.dma_start(out=linwT_b, in_=linwT[ds(512, 1), :])

        lw_sb = [[None] * 4 for _ in range(L)]
        for l in range(L):
            for c in range(4):
                t = consts.tile([P, DIM], f16, tag=f"lw{l}{c}")
                nc.sync.dma_start(out=t, in_=lw2[l, ds(c * P, P), :])
                lw_sb[l][c] = t

        wblk_sb = consts.tile([P, NT * 16 * 32], f16)
        nc.sync.dma_start(out=wblk_sb, in_=wblk[:, :])
        idx_sb = consts.tile([P, NT * P], i16)
        nc.sync.dma_start(out=idx_sb, in_=idxs[:, :])

        # persistent transposed h (fp16), rebuilt each layer
        hT = [consts.tile([P, NSP], f16, tag=f"hT{c}", name=f"hT{c}")
              for c in range(4)]
        h0_col = consts.tile([P, NT], f32)

        def selu_from(x_src, x_src2):
            """selu applied to a [P, F] source (PSUM or SBUF APs).

            x_src/x_src2 are the same values (two APs so PSUM can be read
            by both engines). Returns an SBUF f32 tile of the same free size.
            """
            F = x_src.shape[-1]
            m_t = work.tile([P, F], f32, tag="selu_m")
            nc.vector.tensor_scalar_min(m_t, x_src, 0.0)
            e_t = work.tile([P, F], f32, tag="selu_e")
            nc.scalar.activation(e_t, m_t, ACT.Exp, bias=c_lnla)
            r_t = work.tile([P, F], f32, tag="selu_r")
            nc.scalar.activation(r_t, x_src2, ACT.Relu, scale=LAM)
            s_t = work.tile([P, F], f32, tag="selu_s")
            nc.vector.scalar_tensor_tensor(s_t, e_t, -LA, r_t, A.add, A.add)
            return s_t

        def sqrt_act(out_ap, in_ap, scale, bias_ap):
            """out = sqrt(in*scale + bias) via Exp(0.5*Ln(.)) (one table set)."""
            tmp = small.tile([P, 1], f32, tag="sqrt_tmp")
            nc.scalar.activation(tmp, in_ap, ACT.Ln, scale=scale, bias=bias_ap)
            nc.scalar.activation(out_ap, tmp, ACT.Exp, scale=0.5)

        def transpose_h(h_t, t):
            pt = psum_tp.tile([P, DIM], f32, tag="tp")
            for c in range(4):
                nc.tensor.transpose(pt[:, ds(c * P, P)], h_t[:, ds(c * P, P)], ident)
            for c in range(4):
                nc.vector.tensor_copy(hT[c][:, ts(t, P)], pt[:, ds(c * P, P)])

        # ================= initial phase: linear + selu + exp_map ==========
        for t in range(NT):
            nr_c = []
            for c in range(4):
                tt = nrt_pool.tile([P, P], f16, tag="nr")
                nc.sync.dma_start(out=tt, in_=nrT[ds(c * P, P), ts(t, P)])
                nr_c.append(tt)
            nr_b = nrt_pool.tile([1, P], f16, tag="nrb")
            nc.sync.dma_start(out=nr_b, in_=nrT[ds(512, 1), ts(t, P)])

            pre = psum_mm.tile([P, DIM], f32, tag="mm")
            for c in range(4):
                nc.tensor.matmul(pre, nr_c[c], linwT_sb[c],
                                 start=(c == 0), stop=False)
            nc.tensor.matmul(pre, nr_b, linwT_b, start=False, stop=True)

            v_t = selu_from(pre, pre)

            # exp_map_zero + lorentz normalize
            scr = scr_pool.tile([P, DIM - 1], f32, tag="scr")
            ldv = small.tile([P, 1], f32, tag="ldv")
            nc.scalar.activation(scr, v_t[:, 1:DIM], ACT.Square, accum_out=ldv)
            nd = small.tile([P, 1], f32, tag="nd")
            sqrt_act(nd, ldv, 1.0, c_eps)        # nd = sqrt(ldv + eps)
            t_c = small.tile([P, 1], f32, tag="tc")
            nc.vector.tensor_scalar_min(t_c, nd, 1.0)
            e1 = small.tile([P, 1], f32, tag="e1")
            nc.scalar.activation(e1, t_c, ACT.Exp)
            e2 = small.tile([P, 1], f32, tag="e2")
            nc.scalar.activation(e2, t_c, ACT.Exp, scale=-1.0)
            dd = small.tile([P, 1], f32, tag="dd")
            nc.vector.tensor_sub(dd, e1, e2)
            rn = small.tile([P, 1], f32, tag="rn")
            nc.vector.reciprocal(rn, nd)
            f_c = small.tile([P, 1], f32, tag="fc")
            nc.vector.tensor_scalar(f_c, dd, rn, 0.5, A.mult, A.mult)
            q_c = small.tile([P, 1], f32, tag="qc")
            nc.vector.tensor_scalar(q_c, f_c, f_c, ldv, A.mult, A.mult)
            # h0 = sqrt(1 + f^2 * ldv)
            sqrt_act(h0_col[:, ds(t, 1)], q_c, 1.0, c_one)

            h_t = work2.tile([P, DIM], f32, tag="h")
            nc.scalar.activation(h_t[:, 1:DIM], v_t[:, 1:DIM], ACT.Copy,
                                 scale=f_c)
            nc.vector.tensor_copy(h_t[:, 0:1], h0_col[:, ds(t, 1)])
            transpose_h(h_t, t)

        # ======================= message-passing layers ====================
        for l in range(L):
            zin = dram.tile([NSP, DIM], f16, tag="zin")
            zfull = dram.tile([NFULL, DIM], f16, tag="zfull",
                              addr_space="Shared")

            # ---- phase A: msg GEMM + Klein/z prep ----
            for t in range(NT):
                msg = psum_mm.tile([P, DIM], f32, tag="mm")
                for c in range(4):
                    nc.tensor.matmul(msg, hT[c][:, ts(t, P)], lw_sb[l][c],
                                     start=(c == 0), stop=(c == 3))
                scr = scr_pool.tile([P, DIM - 1], f32, tag="scr")
                ssq = small.tile([P, 1], f32, tag="ssq")
                nc.scalar.activation(scr, msg[:, 1:DIM], ACT.Square,
                                     accum_out=ssq)
                r0 = small.tile([P, 1], f32, tag="r0")
                nc.vector.reciprocal(r0, h0_col[:, ds(t, 1)])
                n2r = small.tile([P, 1], f32, tag="n2r")
                nc.vector.tensor_scalar(n2r, ssq, r0, r0, A.mult, A.mult)
                n2 = small.tile([P, 1], f32, tag="n2")
                nc.vector.tensor_scalar_min(n2, n2r, 0.9)
                # g = 1/sqrt(1-n2) = exp(-0.5*ln(1-n2))
                lg = small.tile([P, 1], f32, tag="lg")
                nc.scalar.activation(lg, n2, ACT.Ln, scale=-1.0, bias=c_one)
                g_c = small.tile([P, 1], f32, tag="gc")
                nc.scalar.activation(g_c, lg, ACT.Exp, scale=-0.5)
                zs = small.tile([P, 1], f32, tag="zs")
                nc.vector.tensor_mul(zs, g_c, r0)
                z_t = zpool.tile([P, DIM], f16, tag="z")
                nc.scalar.activation(z_t[:, 1:DIM], msg[:, 1:DIM], ACT.Copy,
                                     scale=zs)
                nc.vector.tensor_copy(z_t[:, 0:1], g_c)
                nc.sync.dma_start(out=zin[ts(t, P), :], in_=z_t)

            # ---- all-gather of z across the 8 cores ----
            if not sim_mode:
                nc.gpsimd.collective_compute(
                    "AllGather",
                    A.bypass,
                    replica_groups=[list(range(NCORES))],
                    ins=[zin.opt()],
                    outs=[zfull.opt()],
                )

            # ---- phase B: gather + weighted Klein mean + activation ----
            last = l == L - 1
            for t in range(NT):
                g_t = gpool.tile([P, 16, DIM], f16, tag="g")
                for k in range(2):
                    nc.gpsimd.dma_gather(
                        g_t[:, 8 * k:8 * (k + 1), :], zfull[:, :],
                        idx_sb[:, ds(t * P + 64 * k, 64)],
                        1024, 1024, DIM, elem_step=DIM)
                num = psum_nm.tile([P, DIM], f32, tag="num")
                for s in range(4):
                    for a in range(4):
                        gi = 4 * s + a
                        nc.tensor.matmul(
                            num[ds(32 * s, 32), :],
                            wblk_sb[:, ds((t * 16 + gi) * 32, 32)],
                            g_t[:, gi, :],
                            start=(a == 0), stop=(a == 3),
                            tile_position=(0, 32 * s),
                        )
                rn0 = small.tile([P, 1], f32, tag="rn0")
                nc.vector.reciprocal(rn0, num[:, 0:1])
                scr = scr_pool.tile([P, DIM - 1], f32, tag="scr")
                ssn = small.tile([P, 1], f32, tag="ssn")
                nc.scalar.activation(scr, num[:, 1:DIM], ACT.Square,
                                     accum_out=ssn)
                n2r = small.tile([P, 1], f32, tag="n2r")
                nc.vector.tensor_scalar(n2r, ssn, rn0, rn0, A.mult, A.mult)
                n2m = small.tile([P, 1], f32, tag="n2m")
                nc.vector.tensor_scalar_min(n2m, n2r, 0.9)
                lg = small.tile([P, 1], f32, tag="lg")
                nc.scalar.activation(lg, n2m, ACT.Ln, scale=-1.0, bias=c_one)
                g2 = small.tile([P, 1], f32, tag="g2")
                nc.scalar.activation(g2, lg, ACT.Exp, scale=-0.5)
                den = small.tile([P, 1], f32, tag="den")
                nc.vector.tensor_scalar_add(den, g2, 1.0)
                rden = small.tile([P, 1], f32, tag="rden")
                nc.vector.reciprocal(rden, den)
                sxk = small.tile([P, 1], f32, tag="sxk")
                nc.vector.tensor_scalar(sxk, g2, rden, rn0, A.mult, A.mult)
                x_t = work2.tile([P, DIM - 1], f32, tag="x")
                nc.scalar.activation(x_t, num[:, 1:DIM], ACT.Copy, scale=sxk)

                tsel = selu_from(x_t, x_t)

                scr2 = scr_pool.tile([P, DIM - 1], f32, tag="scr")
                ssp = small.tile([P, 1], f32, tag="ssp")
                nc.scalar.activation(scr2, tsel, ACT.Square, accum_out=ssp)
                u1 = small.tile([P, 1], f32, tag="u1")
                nc.vector.tensor_scalar(u1, ssp, -1.0, 1.0 + EPS, A.mult, A.add)
                rp = small.tile([P, 1], f32, tag="rp")
                nc.vector.reciprocal(rp, u1)
                sc2 = small.tile([P, 1], f32, tag="sc2")
                nc.vector.tensor_scalar_mul(sc2, rp, 2.0)
                q_c = small.tile([P, 1], f32, tag="qc2")
                nc.vector.tensor_scalar(q_c, ssp, rp, rp, A.mult, A.mult)
                h_t = work2.tile([P, DIM], f32, tag="h")
                nc.scalar.activation(h_t[:, 1:DIM], tsel, ACT.Copy, scale=sc2)
                # h0 = sqrt(1 + 4*q)
                sqrt_act(h0_col[:, ds(t, 1)], q_c, 4.0, c_one)
                nc.vector.tensor_copy(h_t[:, 0:1], h0_col[:, ds(t, 1)])

                if last:
                    nc.sync.dma_start(out=out_h[ts(t, P), :], in_=h_t)
                else:
                    transpose_h(h_t, t)

    nc.compile()
    return nc


_CACHE = {}


def _get_runner():
    if "runner" in _CACHE:
        return _CACHE["runner"]

    import jax
    from jax.sharding import Mesh, PartitionSpec
    from jax.experimental.shard_map import shard_map
    from concourse import bass2jax

    nc = _build_nc()
    bass2jax.install_neuronx_cc_hook()

    partition_name = (nc.partition_id_tensor.name
                      if nc.partition_id_tensor else None)
    in_names, out_names, out_avals, zero_outs = [], [], [], []
    for alloc in nc.m.functions[0].allocations:
        if not isinstance(alloc, mybir.MemoryLocationSet):
            continue
        name = alloc.memorylocations[0].name
        if alloc.kind == "ExternalInput":
            if name != partition_name:
                in_names.append(name)
        elif alloc.kind == "ExternalOutput":
            out_names.append(name)
            shape = tuple(alloc.tensor_shape)
            dtype = mybir.dt.np(alloc.dtype)
            out_avals.append(jax.core.ShapedArray(shape, dtype))
            zero_outs.append(np.zeros(shape, dtype))
    n_params = len(in_names)
    n_outs = len(out_avals)
    all_names = in_names + out_names
    if partition_name is not None:
        all_names = all_names + [partition_name]

    def _body(*args):
        operands = list(args)
        if partition_name is not None:
            operands.append(bass2jax.partition_id_tensor())
        outs = bass2jax._bass_exec_p.bind(
            *operands,
            out_avals=tuple(out_avals),
            in_names=tuple(all_names),
            out_names=tuple(out_names),
            lowering_input_output_aliases=(),
            sim_require_finite=True,
            sim_require_nnan=True,
            nc=nc,
        )
        return tuple(outs)

    devices = jax.devices()[:NCORES]
    mesh = Mesh(np.asarray(devices), ("core",))
    in_specs = (PartitionSpec("core"),) * (n_params + n_outs)
    out_specs = (PartitionSpec("core"),) * len(out_names)
    donate = tuple(range(n_params, n_params + n_outs))
    sharded = jax.jit(
        shard_map(_body, mesh=mesh, in_specs=in_specs, out_specs=out_specs,
                  check_rep=False),
        donate_argnums=donate, keep_unused=True,
    )

    def runner(in_maps):
        concat_in = [
            np.concatenate([np.asarray(in_maps[c][nm]) for c in range(NCORES)], 0)
            for nm in in_names
        ]
        concat_zero = [
            np.zeros((NCORES * z.shape[0], *z.shape[1:]), z.dtype)
            for z in zero_outs
        ]
        out_arrs = sharded(*concat_in, *concat_zero)
        return [
            {nm: np.asarray(out_arrs[i]).reshape(NCORES, *out_avals[i].shape)[c]
             for i, nm in enumerate(out_names)}
            for c in range(NCORES)
        ]

    _CACHE["runner"] = runner
    return runner


def _prep_inputs(node_repr, adj, weight, lin_w, lin_b, msg_weights):
    """Build the per-core input maps (host-side sharding + relayout)."""
    node_repr = np.asarray(node_repr, np.float32)
    adj = np.asarray(adj, np.int32)
    weight = np.asarray(weight, np.float32)
    lin_w = np.asarray(lin_w, np.float32)
    lin_b = np.asarray(lin_b, np.float32)
    msg_weights = np.asarray(msg_weights, np.float32)

    linwT = np.concatenate([lin_w.T, lin_b[None, :]], 0).astype(np.float16)
    lw2 = np.zeros((L, DIM, DIM), np.float32)
    for l in range(L):
        lw2[l, 0, 0] = 1.0
        lw2[l, 1:, 1:] = msg_weights[l]
    lw2 = lw2.astype(np.float16)

    # global node id -> row in the all-gathered z table
    tbl = ((adj // NS) * NSP + (adj % NS)).astype(np.int32)

    in_maps = []
    for c in range(NCORES):
        sl = slice(c * NS, (c + 1) * NS)
        nr = np.zeros((NSP, DIM), np.float32)
        nr[:NS] = node_repr[sl]
        nrT = np.concatenate([nr.T, np.ones((1, NSP), np.float32)], 0)

        tb = np.zeros((NSP, DEG), np.int32)
        tb[:NS] = tbl[sl]
        w = np.ones((NSP, DEG), np.float32)
        w[:NS] = weight[sl]

        # gather order: tile t, group g=(s,a), slot q=(m,e):
        #   node = t*128 + 32*s + m, neighbor j = 4*a + e
        A5 = tb.reshape(NT, 4, 32, 4, 4)          # [t, s, m, a, e]
        flat = A5.transpose(0, 1, 3, 2, 4).reshape(NT, 16, 128)
        # two gathers of 1024 idxs per tile (HW limit); wrap each half:
        # slot i -> partition i%16, column i//16; replicate across 8 groups
        idx_t = flat.reshape(NT, 2, 1024)
        idx_w = idx_t.reshape(NT, 2, 64, 16).transpose(0, 1, 3, 2)
        idx_core = np.tile(idx_w, (1, 1, 8, 1))       # [t, k, 128, 64]
        idx_dram = idx_core.transpose(2, 0, 1, 3).reshape(
            P, NT * P).astype(np.int16)

        W5 = w.reshape(NT, 4, 32, 4, 4)
        w3 = W5.transpose(0, 1, 3, 2, 4).reshape(NT, 16, 128)
        m_of_q = np.arange(128) // 4
        mask = (m_of_q[:, None] == np.arange(32)[None, :])
        wz = w3[..., None] * mask[None, None]      # [t, g, q, m]
        wblk_dram = wz.transpose(2, 0, 1, 3).reshape(P, NT * 16 * 32).astype(
            np.float16)

        in_maps.append({
            "nrT": nrT.astype(np.float16),
            "linwT": linwT,
            "lw2": lw2,
            "wblk": wblk_dram,
            "idxs": idx_dram,
        })
    return in_maps


def kernel(node_repr, adj, weight, lin_w, lin_b, msg_weights):
    runner = _get_runner()
    in_maps = _prep_inputs(node_repr, adj, weight, lin_w, lin_b, msg_weights)
    results = runner(in_maps)
    out = np.concatenate([results[c]["out_h"][:NS] for c in range(NCORES)], 0)
    return out.astype(np.float32)

